# revision 13
# baseline (speedup 1.0000x reference)
"""Causal single-head attention (B=4, S=2048, d=1024) on 8 TRN2 NeuronCores.

Sharding (uniform single program): core c -> batch b = c//2, subset
s = c%2. Per batch, the 16 query blocks of 128 rows are split into
quads t=0..3; core (b,s) owns blocks {4t+2s, 4t+2s+1}. Every core runs
the identical instruction stream (padded causal limit (t+1)*512 per
quad); the true causal boundary comes from per-core 0/1 mask tiles
supplied as input data.

K/V projections are tensor-parallel within each core pair: core (b,s)
computes the d_out-half s of kT and v for the whole batch; halves are
exchanged with pairwise AllGathers ([[0,1],[2,3],[4,5],[6,7]]).

v5 schedule: K projection first over resident 512-key x chunks (xT
loaded once; first chunks arrive via parallel DMA queues), one 2 MB
K AllGather right after, V projection reusing the resident chunks,
one 2 MB V AllGather, then Q projection (weights/activations ride the
freed x-chunk SBUF slots). Attention computes scores for the late
query group (quads 2,3) then the early group (quads 0,1) as 512-wide
matmuls, then AV per quad in descending t so the tail is the shortest
chain.

Compute (bf16 operands, fp32 PSUM accumulate):
  scoresT[k,q] = kt.T-slices @ qT-group, p = exp(scoresT)
  (no max subtraction: |scores| <= ~2), mask, then
  out[q,:] = (pT.T @ v) / (pT.T @ ones)  -- row sums via ones-matmul.
"""
import sys

sys.path.insert(0, "/opt/trn_rl_repo")

import ml_dtypes
import numpy as np

import concourse.bass as bass  # noqa: F401
import concourse.mybir as mybir
import concourse.tile as tile
from concourse import bacc
from concourse.bass_utils import run_bass_kernel_spmd

B, S, D = 4, 2048, 1024
DC = D // 128          # 8 contraction chunks
NKB = S // 128         # 16 key blocks
SCALE = 1.0 / float(np.sqrt(D))
F32 = mybir.dt.float32
BF = mybir.dt.bfloat16
EXP = mybir.ActivationFunctionType.Exp
GROUPS = [[0, 1], [2, 3], [4, 5], [6, 7]]

_cache = {}


def build_nc():
    nc = bacc.Bacc("TRN2", target_bir_lowering=False, debug=False, num_devices=8)
    # all inputs partition-major: [128, dc, cols]
    xT = nc.dram_tensor("xT", [128, DC, S], BF, kind="ExternalInput")
    xTq = nc.dram_tensor("xTq", [128, DC, 1024], BF, kind="ExternalInput")
    WqT = nc.dram_tensor("WqT", [128, DC, D], BF, kind="ExternalInput")
    WkTh = nc.dram_tensor("WkTh", [128, DC, 512], BF, kind="ExternalInput")
    WvTh = nc.dram_tensor("WvTh", [128, DC, 512], BF, kind="ExternalInput")
    masks = nc.dram_tensor("masks", [128, 4, 256], BF, kind="ExternalInput")
    out = nc.dram_tensor("out", [1024, D], F32, kind="ExternalOutput")
    kg_in = nc.dram_tensor("kg_in", [128, 4, S], BF)
    kg_out = nc.dram_tensor("kg_out", [2, 128, 4, S], BF)
    vg_in = nc.dram_tensor("vg_in", [128, NKB, 512], BF)
    vg_out = nc.dram_tensor("vg_out", [2, 128, NKB, 512], BF)

    with tile.TileContext(nc) as tc:
        with (
            tc.tile_pool(name="w", bufs=1) as wp,
            tc.tile_pool(name="per", bufs=1) as per,
            tc.tile_pool(name="px", bufs=4) as pxp,
            tc.tile_pool(name="stg", bufs=2) as stg,
            tc.tile_pool(name="pt5", bufs=16) as ptp5,
            tc.tile_pool(name="pt2", bufs=8) as ptp2,
            tc.tile_pool(name="ot", bufs=4) as otp,
            tc.tile_pool(name="sml", bufs=4) as smlp,
            tc.tile_pool(name="mix", bufs=5, space="PSUM") as mixp,
            tc.tile_pool(name="psav", bufs=3, space="PSUM") as psavp,
        ):
            # ---------------- consts + persistent ----------------
            kt = per.tile([128, DC, S], BF)        # kT: [d_out, 2048]
            vv = per.tile([128, 2, NKB, 512], BF)  # v: [2048, (rank0|rank1) 512]
            qt = per.tile([128, DC, 1024], BF)     # qT: [d_out, 1024]
            zeros_f = per.tile([128, 2], F32)
            ones = per.tile([128, 2], BF)
            maskt = per.tile([128, 4, 256], BF)
            nc.vector.memset(zeros_f, 0.0)
            # exp(0)=1 -> also preloads the ACT exp table long before attention
            nc.scalar.activation(ones, zeros_f, EXP)

            wk = wp.tile([128, DC, 512], BF)
            wv = wp.tile([128, DC, 512], BF)
            # startup: wk on sync while xk0/xk1 arrive via the gpsimd queue,
            # so the first matmul chain starts as early as possible.
            nc.sync.dma_start(out=wk, in_=WkTh[:])
            xks = []
            for sc in range(4):
                xk = pxp.tile([128, DC, 512], BF, tag="xs", name=f"xk_{sc}")
                eng = nc.gpsimd if sc < 2 else nc.sync
                eng.dma_start(out=xk, in_=xT[:, :, sc * 512:(sc + 1) * 512])
                xks.append(xk)
            # scalar queue: V weights + masks up front
            nc.scalar.dma_start(out=wv, in_=WvTh[:])
            nc.scalar.dma_start(out=maskt, in_=masks[:])
            # Q-projection inputs ride the freed x-chunk slots (ring order:
            # wqa evicts xk0, xqa evicts xk1, wqb evicts xk2, xqb evicts xk3)
            wqs, xqs = [], []
            for h in range(2):
                wqh = pxp.tile([128, DC, 512], BF, tag="xs", name=f"wq_{h}")
                nc.sync.dma_start(out=wqh, in_=WqT[:, :, h * 512:(h + 1) * 512])
                wqs.append(wqh)
                xqh = pxp.tile([128, DC, 512], BF, tag="xs", name=f"xq_{h}")
                nc.sync.dma_start(out=xqh, in_=xTq[:, :, h * 512:(h + 1) * 512])
                xqs.append(xqh)

            # -------- K half-projection -> one 2MB AllGather --------
            for g in range(2):
                kgs = stg.tile([128, 4, 1024], BF, tag="kgs", name=f"kgs_{g}")
                for scl in range(2):
                    xk = xks[2 * g + scl]
                    for ocl in range(4):
                        ps = mixp.tile([128, 512], F32, tag="mix")
                        for dc in range(DC):
                            nc.tensor.matmul(
                                ps,
                                lhsT=wk[:, dc, ocl * 128:(ocl + 1) * 128],
                                rhs=xk[:, dc, :],
                                start=(dc == 0),
                                stop=(dc == DC - 1),
                            )
                        nc.vector.tensor_copy(
                            kgs[:, ocl, scl * 512:(scl + 1) * 512], ps
                        )
                nc.scalar.dma_start(
                    out=kg_in[:, :, g * 1024:(g + 1) * 1024], in_=kgs
                )
            nc.gpsimd.collective_compute(
                "AllGather",
                mybir.AluOpType.bypass,
                replica_groups=GROUPS,
                ins=[kg_in[:]],
                outs=[kg_out[:]],
            )

            # -------- V half-projection (resident x) -> one 2MB AllGather ---
            for g in range(2):
                vgs = stg.tile([128, 8, 512], BF, tag="vgs", name=f"vgs_{g}")
                for scl in range(2):
                    xk = xks[2 * g + scl]
                    for sb in range(4):
                        ps = mixp.tile([128, 512], F32, tag="mix",
                                       name=f"ps2_{g}_{scl}_{sb}")
                        for dc in range(DC):
                            nc.tensor.matmul(
                                ps,
                                lhsT=xk[:, dc, sb * 128:(sb + 1) * 128],
                                rhs=wv[:, dc, :],
                                start=(dc == 0),
                                stop=(dc == DC - 1),
                            )
                        nc.vector.tensor_copy(vgs[:, scl * 4 + sb, :], ps)
                nc.scalar.dma_start(
                    out=vg_in[:, g * 8:(g + 1) * 8, :], in_=vgs
                )
            nc.gpsimd.collective_compute(
                "AllGather",
                mybir.AluOpType.bypass,
                replica_groups=GROUPS,
                ins=[vg_in[:]],
                outs=[vg_out[:]],
            )

            # gathered K/V -> SBUF
            for r in range(2):
                nc.sync.dma_start(out=kt[:, r * 4:(r + 1) * 4, :], in_=kg_out[r])
            for r in range(2):
                nc.scalar.dma_start(out=vv[:, r, :, :], in_=vg_out[r])

            # -------- Q projection -> qt (sc outer: xqb may arrive late) ----
            for sc in range(2):
                for oc in range(8):
                    ps = mixp.tile([128, 512], F32, tag="mix",
                                   name=f"ps0_{sc}_{oc}")
                    for dc in range(DC):
                        nc.tensor.matmul(
                            ps,
                            lhsT=wqs[oc // 4][:, dc, (oc % 4) * 128:
                                              (oc % 4 + 1) * 128],
                            rhs=xqs[sc][:, dc, :],
                            start=(dc == 0),
                            stop=(dc == DC - 1),
                        )
                    nc.vector.tensor_copy(
                        qt[:, oc, sc * 512:(sc + 1) * 512], ps
                    )

            # ---------------- attention ----------------
            # scores per 512-query group (quad pair); pt tile lookup:
            # pt_of[(t, kb)] -> (tile, column offset of quad t's 256 cols)
            pt_of = {}

            def emit_scores_group(tlo):
                # group = quads (tlo, tlo+1), query cols [512*tlo/2 ...]
                qbase = tlo * 256
                Lfull = 4 * tlo + 4   # kb limit for the low quad
                Lhigh = 4 * tlo + 8   # kb limit for the high quad
                for kb in range(Lfull):
                    ps = mixp.tile([128, 512], F32, tag="mix",
                                   name=f"s5_{tlo}_{kb}")
                    for dc in range(DC):
                        nc.tensor.matmul(
                            ps,
                            lhsT=kt[:, dc, kb * 128:(kb + 1) * 128],
                            rhs=qt[:, dc, qbase:qbase + 512],
                            start=(dc == 0),
                            stop=(dc == DC - 1),
                        )
                    pt = ptp5.tile([128, 512], BF, tag="pt5",
                                   name=f"p5_{tlo}_{kb}")
                    nc.scalar.activation(pt, ps, EXP)
                    kbr = kb - 4 * tlo
                    if kbr >= 0:
                        nc.vector.tensor_mul(
                            pt[:, 0:256], pt[:, 0:256], maskt[:, kbr, :]
                        )
                    pt_of[(tlo, kb)] = (pt, 0)
                    pt_of[(tlo + 1, kb)] = (pt, 256)
                for kb in range(Lfull, Lhigh):
                    ps = mixp.tile([128, 512], F32, tag="mix",
                                   name=f"s2_{tlo}_{kb}")
                    for dc in range(DC):
                        nc.tensor.matmul(
                            ps[:, 0:256],
                            lhsT=kt[:, dc, kb * 128:(kb + 1) * 128],
                            rhs=qt[:, dc, qbase + 256:qbase + 512],
                            start=(dc == 0),
                            stop=(dc == DC - 1),
                        )
                    pt = ptp2.tile([128, 256], BF, tag="pt2",
                                   name=f"p2_{tlo}_{kb}")
                    nc.scalar.activation(pt, ps[:, 0:256], EXP)
                    nc.vector.tensor_mul(pt, pt, maskt[:, kb - Lfull, :])
                    pt_of[(tlo + 1, kb)] = (pt, 0)

            def emit_av(t):
                L = 4 * t + 4
                recs, ots = [], []
                for j in range(2):
                    lps = psavp.tile([128, 2], F32, tag="psav", name=f"l_{t}_{j}")
                    for kb in range(L):
                        pt, off = pt_of[(t, kb)]
                        nc.tensor.matmul(
                            lps,
                            lhsT=pt[:, off + j * 128:off + (j + 1) * 128],
                            rhs=ones,
                            start=(kb == 0),
                            stop=(kb == L - 1),
                        )
                    rec = smlp.tile([128, 1], F32, tag="rec")
                    nc.vector.reciprocal(rec, lps[:, 0:1])
                    recs.append(rec)
                for j in range(2):
                    for oh in range(2):
                        avp = psavp.tile([128, 512], F32, tag="psav",
                                         name=f"av_{t}_{j}_{oh}")
                        for kb in range(L):
                            pt, off = pt_of[(t, kb)]
                            nc.tensor.matmul(
                                avp,
                                lhsT=pt[:, off + j * 128:off + (j + 1) * 128],
                                rhs=vv[:, oh, kb, :],
                                start=(kb == 0),
                                stop=(kb == L - 1),
                            )
                        ot = otp.tile([128, 512], F32, tag="ot",
                                      name=f"ot_{t}_{j}_{oh}")
                        nc.vector.tensor_scalar_mul(ot, avp, recs[j])
                        nc.sync.dma_start(
                            out=out[t * 256 + j * 128: t * 256 + (j + 1) * 128,
                                    oh * 512:(oh + 1) * 512],
                            in_=ot,
                        )

            emit_scores_group(2)   # quads 2,3 (needs all of kt)
            emit_scores_group(0)   # quads 0,1
            emit_av(3)
            emit_av(2)
            emit_av(1)
            emit_av(0)
    nc.compile()
    return nc


def _query_cols(sub):
    return np.concatenate(
        [
            np.arange((4 * t + 2 * sub) * 128, (4 * t + 2 * sub + 2) * 128)
            for t in range(4)
        ]
    )


def _masks(sub):
    m = np.zeros((4, 128, 256), np.float32)
    p = np.arange(128)[:, None]
    j = np.arange(256)[None, :]
    qoff = (2 * sub + j // 128) * 128 + j % 128
    for kbr in range(4):
        m[kbr] = (kbr * 128 + p <= qoff).astype(np.float32)
    return np.ascontiguousarray(m.transpose(1, 0, 2))  # -> [128, 4, 256]


def _pmaj(a):
    """[dc*128, cols] -> partition-major [128, dc, cols]."""
    d, cols = a.shape
    return np.ascontiguousarray(a.reshape(d // 128, 128, cols).transpose(1, 0, 2))


def kernel(x, Wq, Wk, Wv, _trace=False):
    if "nc" not in _cache:
        _cache["nc"] = build_nc()
    nc = _cache["nc"]

    bf = ml_dtypes.bfloat16
    x = np.asarray(x, dtype=np.float32)
    WqT = _pmaj((np.asarray(Wq, np.float32).T * np.float32(SCALE)).astype(bf))
    WkT = np.asarray(Wk, np.float32).T.astype(bf)
    WvT = np.asarray(Wv, np.float32).T.astype(bf)

    in_maps = []
    for c in range(8):
        b, sub = c // 2, c % 2
        xT = x[b].T.astype(bf)
        in_maps.append(
            {
                "xT": _pmaj(xT),
                "xTq": _pmaj(np.ascontiguousarray(xT[:, _query_cols(sub)])),
                "WqT": WqT,
                "WkTh": _pmaj(WkT[:, sub * 512:(sub + 1) * 512]),
                "WvTh": _pmaj(WvT[:, sub * 512:(sub + 1) * 512]),
                "masks": _masks(sub).astype(bf),
            }
        )

    res = run_bass_kernel_spmd(
        nc, in_maps, core_ids=list(range(8)), trace=_trace
    )
    full = np.empty((B, S, D), np.float32)
    for c in range(8):
        b, sub = c // 2, c % 2
        full[b, _query_cols(sub)] = res.results[c]["out"]
    if _trace:
        _cache["last_result"] = res
    return full


# revision 17
# speedup vs baseline: 1.0147x; 1.0147x over previous
"""Causal single-head attention (B=4, S=2048, d=1024) on 8 TRN2 NeuronCores.

Sharding (uniform single program): core c -> batch b = c//2, subset
s = c%2. Per batch, the 16 query blocks of 128 rows are split into
quads t=0..3; core (b,s) owns blocks {4t+2s, 4t+2s+1}. Every core runs
the identical instruction stream (padded causal limit (t+1)*512 per
quad); the true causal boundary comes from per-core 0/1 mask tiles
supplied as input data.

K/V projections are tensor-parallel within each core pair: core (b,s)
computes the d_out-half s of kT and v for the whole batch; halves are
exchanged with pairwise AllGathers ([[0,1],[2,3],[4,5],[6,7]]).

v5 schedule: K projection first over resident 512-key x chunks (xT
loaded once; first chunks arrive via parallel DMA queues), one 2 MB
K AllGather right after, V projection reusing the resident chunks,
one 2 MB V AllGather, then Q projection (weights/activations ride the
freed x-chunk SBUF slots). Attention computes scores for the late
query group (quads 2,3) then the early group (quads 0,1) as 512-wide
matmuls, then AV per quad in descending t so the tail is the shortest
chain.

Compute (bf16 operands, fp32 PSUM accumulate):
  scoresT[k,q] = kt.T-slices @ qT-group, p = exp(scoresT)
  (no max subtraction: |scores| <= ~2), mask, then
  out[q,:] = (pT.T @ v) / (pT.T @ ones)  -- row sums via ones-matmul.
"""
import sys

sys.path.insert(0, "/opt/trn_rl_repo")

import ml_dtypes
import numpy as np

import concourse.bass as bass  # noqa: F401
import concourse.mybir as mybir
import concourse.tile as tile
from concourse import bacc
from concourse.bass_utils import run_bass_kernel_spmd

B, S, D = 4, 2048, 1024
DC = D // 128          # 8 contraction chunks
NKB = S // 128         # 16 key blocks
SCALE = 1.0 / float(np.sqrt(D))
F32 = mybir.dt.float32
BF = mybir.dt.bfloat16
EXP = mybir.ActivationFunctionType.Exp
GROUPS = [[0, 1], [2, 3], [4, 5], [6, 7]]

_cache = {}


def build_nc():
    nc = bacc.Bacc("TRN2", target_bir_lowering=False, debug=False, num_devices=8)
    # all inputs partition-major: [128, dc, cols]
    xT = nc.dram_tensor("xT", [128, DC, S], BF, kind="ExternalInput")
    xTq = nc.dram_tensor("xTq", [128, DC, 1024], BF, kind="ExternalInput")
    WqT = nc.dram_tensor("WqT", [128, DC, D], BF, kind="ExternalInput")
    WkTh = nc.dram_tensor("WkTh", [128, DC, 512], BF, kind="ExternalInput")
    WvTh = nc.dram_tensor("WvTh", [128, DC, 512], BF, kind="ExternalInput")
    masks = nc.dram_tensor("masks", [128, 4, 256], BF, kind="ExternalInput")
    out = nc.dram_tensor("out", [1024, D], F32, kind="ExternalOutput")
    kg_in = nc.dram_tensor("kg_in", [2, 128, 4, 1024], BF)
    kg_out = nc.dram_tensor("kg_out", [2, 2, 128, 4, 1024], BF)
    vg_in = nc.dram_tensor("vg_in", [2, 128, 8, 512], BF)
    vg_out = nc.dram_tensor("vg_out", [2, 2, 128, 8, 512], BF)
    warm_in = nc.dram_tensor("warm_in", [1, 64], BF)
    warm_out = nc.dram_tensor("warm_out", [2, 64], BF)

    with tile.TileContext(nc) as tc:
        with (
            tc.tile_pool(name="w", bufs=1) as wp,
            tc.tile_pool(name="per", bufs=1) as per,
            tc.tile_pool(name="px", bufs=4) as pxp,
            tc.tile_pool(name="stg", bufs=2) as stg,
            tc.tile_pool(name="pt5", bufs=16) as ptp5,
            tc.tile_pool(name="pt2", bufs=8) as ptp2,
            tc.tile_pool(name="ot", bufs=4) as otp,
            tc.tile_pool(name="sml", bufs=4) as smlp,
            tc.tile_pool(name="mix", bufs=5, space="PSUM") as mixp,
            tc.tile_pool(name="psav", bufs=3, space="PSUM") as psavp,
        ):
            # ---------------- consts + persistent ----------------
            kt = per.tile([128, DC, S], BF)        # kT: [d_out, 2048]
            vv = per.tile([128, 2, NKB, 512], BF)  # v: [2048, (rank0|rank1) 512]
            qt = per.tile([128, DC, 1024], BF)     # qT: [d_out, 1024]
            zeros_f = per.tile([128, 2], F32)
            ones = per.tile([128, 2], BF)
            maskt = per.tile([128, 4, 256], BF)
            nc.vector.memset(zeros_f, 0.0)
            # exp(0)=1 -> also preloads the ACT exp table long before attention
            nc.scalar.activation(ones, zeros_f, EXP)

            # tiny warmup AllGather (garbage data, nobody reads it): absorbs
            # the one-time collective-stream barrier + start delay at t=0
            nc.gpsimd.collective_compute(
                "AllGather",
                mybir.AluOpType.bypass,
                replica_groups=GROUPS,
                ins=[warm_in[:]],
                outs=[warm_out[:]],
            )

            wk = wp.tile([128, DC, 512], BF)
            wv = wp.tile([128, DC, 512], BF)
            # startup: wk on sync while xk0/xk1 arrive via the gpsimd queue,
            # so the first matmul chain starts as early as possible.
            nc.sync.dma_start(out=wk, in_=WkTh[:])
            xks = []
            for sc in range(4):
                xk = pxp.tile([128, DC, 512], BF, tag="xs", name=f"xk_{sc}")
                eng = nc.gpsimd if sc < 2 else nc.sync
                eng.dma_start(out=xk, in_=xT[:, :, sc * 512:(sc + 1) * 512])
                xks.append(xk)
            # scalar queue: V weights + masks up front
            nc.scalar.dma_start(out=wv, in_=WvTh[:])
            nc.scalar.dma_start(out=maskt, in_=masks[:])
            # Q-projection inputs ride the freed x-chunk slots (ring order:
            # wqa evicts xk0, xqa evicts xk1, wqb evicts xk2, xqb evicts xk3)
            wqs, xqs = [], []
            for h in range(2):
                wqh = pxp.tile([128, DC, 512], BF, tag="xs", name=f"wq_{h}")
                nc.sync.dma_start(out=wqh, in_=WqT[:, :, h * 512:(h + 1) * 512])
                wqs.append(wqh)
                xqh = pxp.tile([128, DC, 512], BF, tag="xs", name=f"xq_{h}")
                nc.sync.dma_start(out=xqh, in_=xTq[:, :, h * 512:(h + 1) * 512])
                xqs.append(xqh)

            # -------- K half-projection -> two 1MB AllGathers --------
            for g in range(2):
                kgs = stg.tile([128, 4, 1024], BF, tag="kgs", name=f"kgs_{g}")
                for scl in range(2):
                    xk = xks[2 * g + scl]
                    for ocl in range(4):
                        ps = mixp.tile([128, 512], F32, tag="mix")
                        for dc in range(DC):
                            nc.tensor.matmul(
                                ps,
                                lhsT=wk[:, dc, ocl * 128:(ocl + 1) * 128],
                                rhs=xk[:, dc, :],
                                start=(dc == 0),
                                stop=(dc == DC - 1),
                            )
                        nc.vector.tensor_copy(
                            kgs[:, ocl, scl * 512:(scl + 1) * 512], ps
                        )
                nc.scalar.dma_start(out=kg_in[g], in_=kgs)
                nc.gpsimd.collective_compute(
                    "AllGather",
                    mybir.AluOpType.bypass,
                    replica_groups=GROUPS,
                    ins=[kg_in[g]],
                    outs=[kg_out[g]],
                )
                # gathered K group -> SBUF; the gpsimd queue blocks on the
                # gather's completion, which is harmless (the next trigger's
                # own input is not ready earlier anyway)
                for r in range(2):
                    nc.gpsimd.dma_start(
                        out=kt[:, r * 4:(r + 1) * 4, g * 1024:(g + 1) * 1024],
                        in_=kg_out[g, r],
                    )

            # -------- V half-projection (resident x) -> two 1MB AllGathers --
            for g in range(2):
                vgs = stg.tile([128, 8, 512], BF, tag="vgs", name=f"vgs_{g}")
                for scl in range(2):
                    xk = xks[2 * g + scl]
                    for sb in range(4):
                        ps = mixp.tile([128, 512], F32, tag="mix",
                                       name=f"ps2_{g}_{scl}_{sb}")
                        for dc in range(DC):
                            nc.tensor.matmul(
                                ps,
                                lhsT=xk[:, dc, sb * 128:(sb + 1) * 128],
                                rhs=wv[:, dc, :],
                                start=(dc == 0),
                                stop=(dc == DC - 1),
                            )
                        nc.vector.tensor_copy(vgs[:, scl * 4 + sb, :], ps)
                nc.scalar.dma_start(out=vg_in[g], in_=vgs)
                nc.gpsimd.collective_compute(
                    "AllGather",
                    mybir.AluOpType.bypass,
                    replica_groups=GROUPS,
                    ins=[vg_in[g]],
                    outs=[vg_out[g]],
                )
                for r in range(2):
                    nc.scalar.dma_start(
                        out=vv[:, r, 8 * g:8 * g + 8, :], in_=vg_out[g, r]
                    )

            # -------- Q projection -> qt (sc outer: xqb may arrive late) ----
            for sc in range(2):
                for oc in range(8):
                    ps = mixp.tile([128, 512], F32, tag="mix",
                                   name=f"ps0_{sc}_{oc}")
                    for dc in range(DC):
                        nc.tensor.matmul(
                            ps,
                            lhsT=wqs[oc // 4][:, dc, (oc % 4) * 128:
                                              (oc % 4 + 1) * 128],
                            rhs=xqs[sc][:, dc, :],
                            start=(dc == 0),
                            stop=(dc == DC - 1),
                        )
                    nc.vector.tensor_copy(
                        qt[:, oc, sc * 512:(sc + 1) * 512], ps
                    )

            # ---------------- attention ----------------
            # scores per 512-query group (quad pair); pt tile lookup:
            # pt_of[(t, kb)] -> (tile, column offset of quad t's 256 cols)
            pt_of = {}

            def emit_scores_group(tlo):
                # group = quads (tlo, tlo+1), query cols [512*tlo/2 ...]
                qbase = tlo * 256
                Lfull = 4 * tlo + 4   # kb limit for the low quad
                Lhigh = 4 * tlo + 8   # kb limit for the high quad
                for kb in range(Lfull):
                    ps = mixp.tile([128, 512], F32, tag="mix",
                                   name=f"s5_{tlo}_{kb}")
                    for dc in range(DC):
                        nc.tensor.matmul(
                            ps,
                            lhsT=kt[:, dc, kb * 128:(kb + 1) * 128],
                            rhs=qt[:, dc, qbase:qbase + 512],
                            start=(dc == 0),
                            stop=(dc == DC - 1),
                        )
                    pt = ptp5.tile([128, 512], BF, tag="pt5",
                                   name=f"p5_{tlo}_{kb}")
                    nc.scalar.activation(pt, ps, EXP)
                    kbr = kb - 4 * tlo
                    if kbr >= 0:
                        nc.vector.tensor_mul(
                            pt[:, 0:256], pt[:, 0:256], maskt[:, kbr, :]
                        )
                    pt_of[(tlo, kb)] = (pt, 0)
                    pt_of[(tlo + 1, kb)] = (pt, 256)
                for kb in range(Lfull, Lhigh):
                    ps = mixp.tile([128, 512], F32, tag="mix",
                                   name=f"s2_{tlo}_{kb}")
                    for dc in range(DC):
                        nc.tensor.matmul(
                            ps[:, 0:256],
                            lhsT=kt[:, dc, kb * 128:(kb + 1) * 128],
                            rhs=qt[:, dc, qbase + 256:qbase + 512],
                            start=(dc == 0),
                            stop=(dc == DC - 1),
                        )
                    pt = ptp2.tile([128, 256], BF, tag="pt2",
                                   name=f"p2_{tlo}_{kb}")
                    nc.scalar.activation(pt, ps[:, 0:256], EXP)
                    nc.vector.tensor_mul(pt, pt, maskt[:, kb - Lfull, :])
                    pt_of[(tlo + 1, kb)] = (pt, 0)

            def emit_av(t):
                L = 4 * t + 4
                recs, ots = [], []
                for j in range(2):
                    lps = psavp.tile([128, 2], F32, tag="psav", name=f"l_{t}_{j}")
                    for kb in range(L):
                        pt, off = pt_of[(t, kb)]
                        nc.tensor.matmul(
                            lps,
                            lhsT=pt[:, off + j * 128:off + (j + 1) * 128],
                            rhs=ones,
                            start=(kb == 0),
                            stop=(kb == L - 1),
                        )
                    rec = smlp.tile([128, 1], F32, tag="rec")
                    nc.vector.reciprocal(rec, lps[:, 0:1])
                    recs.append(rec)
                for j in range(2):
                    for oh in range(2):
                        avp = psavp.tile([128, 512], F32, tag="psav",
                                         name=f"av_{t}_{j}_{oh}")
                        for kb in range(L):
                            pt, off = pt_of[(t, kb)]
                            nc.tensor.matmul(
                                avp,
                                lhsT=pt[:, off + j * 128:off + (j + 1) * 128],
                                rhs=vv[:, oh, kb, :],
                                start=(kb == 0),
                                stop=(kb == L - 1),
                            )
                        ot = otp.tile([128, 512], F32, tag="ot",
                                      name=f"ot_{t}_{j}_{oh}")
                        nc.vector.tensor_scalar_mul(ot, avp, recs[j])
                        nc.sync.dma_start(
                            out=out[t * 256 + j * 128: t * 256 + (j + 1) * 128,
                                    oh * 512:(oh + 1) * 512],
                            in_=ot,
                        )

            # group 0 first: it only needs the first K gather group, giving
            # the second K group and the V gathers extra slack
            emit_scores_group(0)   # quads 0,1 (kb < 8)
            emit_scores_group(2)   # quads 2,3 (all of kt)
            emit_av(3)
            emit_av(2)
            emit_av(1)
            emit_av(0)
    nc.compile()
    return nc


def _query_cols(sub):
    return np.concatenate(
        [
            np.arange((4 * t + 2 * sub) * 128, (4 * t + 2 * sub + 2) * 128)
            for t in range(4)
        ]
    )


def _masks(sub):
    m = np.zeros((4, 128, 256), np.float32)
    p = np.arange(128)[:, None]
    j = np.arange(256)[None, :]
    qoff = (2 * sub + j // 128) * 128 + j % 128
    for kbr in range(4):
        m[kbr] = (kbr * 128 + p <= qoff).astype(np.float32)
    return np.ascontiguousarray(m.transpose(1, 0, 2))  # -> [128, 4, 256]


def _pmaj(a):
    """[dc*128, cols] -> partition-major [128, dc, cols]."""
    d, cols = a.shape
    return np.ascontiguousarray(a.reshape(d // 128, 128, cols).transpose(1, 0, 2))


def kernel(x, Wq, Wk, Wv, _trace=False):
    if "nc" not in _cache:
        _cache["nc"] = build_nc()
    nc = _cache["nc"]

    bf = ml_dtypes.bfloat16
    x = np.asarray(x, dtype=np.float32)
    WqT = _pmaj((np.asarray(Wq, np.float32).T * np.float32(SCALE)).astype(bf))
    WkT = np.asarray(Wk, np.float32).T.astype(bf)
    WvT = np.asarray(Wv, np.float32).T.astype(bf)

    in_maps = []
    for c in range(8):
        b, sub = c // 2, c % 2
        xT = x[b].T.astype(bf)
        in_maps.append(
            {
                "xT": _pmaj(xT),
                "xTq": _pmaj(np.ascontiguousarray(xT[:, _query_cols(sub)])),
                "WqT": WqT,
                "WkTh": _pmaj(WkT[:, sub * 512:(sub + 1) * 512]),
                "WvTh": _pmaj(WvT[:, sub * 512:(sub + 1) * 512]),
                "masks": _masks(sub).astype(bf),
            }
        )

    res = run_bass_kernel_spmd(
        nc, in_maps, core_ids=list(range(8)), trace=_trace
    )
    full = np.empty((B, S, D), np.float32)
    for c in range(8):
        b, sub = c // 2, c % 2
        full[b, _query_cols(sub)] = res.results[c]["out"]
    if _trace:
        _cache["last_result"] = res
    return full


# revision 20
# speedup vs baseline: 1.1706x; 1.1537x over previous
"""Causal single-head attention (B=4, S=2048, d=1024) on 8 TRN2 NeuronCores.

Sharding (uniform single program): core c -> batch b = c//2, subset
s = c%2. Per batch, the 16 query blocks of 128 rows are split into
quads t=0..3; core (b,s) owns blocks {4t+2s, 4t+2s+1}. Every core runs
the identical instruction stream (padded causal limit (t+1)*512 per
quad); the true causal boundary comes from per-core 0/1 mask tiles
supplied as input data.

K/V projections are tensor-parallel within each core pair: core (b,s)
computes the d_out-half s of kT and v for the whole batch; halves are
exchanged with pairwise AllGathers ([[0,1],[2,3],[4,5],[6,7]]).

v7 schedule: a tiny warmup collective absorbs the one-time collective
barrier/start delay. K projection streams over resident x chunks
(loaded once via three parallel DMA queues); each 1024-key half is
AllGathered (1 MB) as soon as it completes. V projection reuses the
resident chunks with two more 1 MB gathers. All Q-projection inputs
load before the gather window so the link sees minimal contention.
Gathered K/V land in per-group SBUF tiles (fine-grained readiness).
Attention: scores for quads 0,1 (needs only K group 0), then quads
2,3, then AV per quad in descending t so the tail is the shortest
chain. Score matmuls are 512 queries wide where the causal structure
allows.

Compute (bf16 operands, fp32 PSUM accumulate):
  scoresT[k,q] = kt.T-slices @ qT-group, p = exp(scoresT)
  (no max subtraction: |scores| <= ~2), mask, then
  out[q,:] = (pT.T @ v) / (pT.T @ ones)  -- row sums via ones-matmul.
"""
import sys

sys.path.insert(0, "/opt/trn_rl_repo")

import ml_dtypes
import numpy as np

import concourse.bass as bass  # noqa: F401
import concourse.mybir as mybir
import concourse.tile as tile
from concourse import bacc
from concourse.bass_utils import run_bass_kernel_spmd

B, S, D = 4, 2048, 1024
DC = D // 128          # 8 contraction chunks
NKB = S // 128         # 16 key blocks
SCALE = 1.0 / float(np.sqrt(D))
F32 = mybir.dt.float32
BF = mybir.dt.bfloat16
EXP = mybir.ActivationFunctionType.Exp
GROUPS = [[0, 1], [2, 3], [4, 5], [6, 7]]

_cache = {}


def build_nc():
    nc = bacc.Bacc("TRN2", target_bir_lowering=False, debug=False, num_devices=8)
    # all inputs partition-major: [128, dc, cols]
    xT = nc.dram_tensor("xT", [128, DC, S], BF, kind="ExternalInput")
    xTq = nc.dram_tensor("xTq", [128, DC, 1024], BF, kind="ExternalInput")
    WqT = nc.dram_tensor("WqT", [128, DC, D], BF, kind="ExternalInput")
    WkTh = nc.dram_tensor("WkTh", [128, DC, 512], BF, kind="ExternalInput")
    WvTh = nc.dram_tensor("WvTh", [128, DC, 512], BF, kind="ExternalInput")
    masks = nc.dram_tensor("masks", [128, 4, 256], BF, kind="ExternalInput")
    out = nc.dram_tensor("out", [1024, D], F32, kind="ExternalOutput")
    kg_in = nc.dram_tensor("kg_in", [2, 128, 4, 1024], BF)
    kg_out = nc.dram_tensor("kg_out", [2, 2, 128, 4, 1024], BF)
    vg_in = nc.dram_tensor("vg_in", [2, 128, 8, 512], BF)
    vg_out = nc.dram_tensor("vg_out", [2, 2, 128, 8, 512], BF)
    warm_in = nc.dram_tensor("warm_in", [1, 64], BF)
    warm_out = nc.dram_tensor("warm_out", [2, 64], BF)

    with tile.TileContext(nc) as tc:
        with (
            tc.tile_pool(name="w", bufs=1) as wp,
            tc.tile_pool(name="per", bufs=1) as per,
            tc.tile_pool(name="px", bufs=4) as pxp,
            tc.tile_pool(name="stg", bufs=2) as stg,
            tc.tile_pool(name="pt5", bufs=16) as ptp5,
            tc.tile_pool(name="pt2", bufs=8) as ptp2,
            tc.tile_pool(name="ot", bufs=2) as otp,
            tc.tile_pool(name="sml", bufs=4) as smlp,
            tc.tile_pool(name="mix", bufs=5, space="PSUM") as mixp,
            tc.tile_pool(name="psav", bufs=3, space="PSUM") as psavp,
        ):
            # tiny warmup AllGather (garbage data, nobody reads it): absorbs
            # the one-time collective-stream barrier + start delay
            nc.gpsimd.collective_compute(
                "AllGather",
                mybir.AluOpType.bypass,
                replica_groups=GROUPS,
                ins=[warm_in[:]],
                outs=[warm_out[:]],
            )

            # ---------------- consts + persistent ----------------
            # kt/vv split per 1024-key gather group for fine-grained readiness
            kts = [per.tile([128, DC, 1024], BF, name=f"kt{g}") for g in range(2)]
            vvs = [per.tile([128, 2, 8, 512], BF, name=f"vv{g}") for g in range(2)]
            qt = per.tile([128, DC, 1024], BF)     # qT: [d_out, 1024]
            zeros_f = per.tile([128, 2], F32)
            ones = per.tile([128, 2], BF)
            maskt = per.tile([128, 4, 256], BF)
            nc.vector.memset(zeros_f, 0.0)
            # exp(0)=1 -> also preloads the ACT exp table long before attention
            nc.scalar.activation(ones, zeros_f, EXP)

            wk = wp.tile([128, DC, 512], BF)
            wv = wp.tile([128, DC, 512], BF)
            wq = wp.tile([128, DC, D], BF)
            xq = wp.tile([128, DC, 1024], BF)
            # startup: wk halves on sync, xk0 on gpsimd, xk1 on scalar --
            # three parallel ~125GB/s queues so the K chain starts early.
            nc.sync.dma_start(out=wk[:, 0:4, :], in_=WkTh[:, 0:4, :])
            nc.sync.dma_start(out=wk[:, 4:8, :], in_=WkTh[:, 4:8, :])
            xks = []
            for sc in range(4):
                xk = pxp.tile([128, DC, 512], BF, tag="xs", name=f"xk_{sc}")
                eng = (nc.gpsimd, nc.scalar, nc.sync, nc.sync)[sc]
                eng.dma_start(out=xk, in_=xT[:, :, sc * 512:(sc + 1) * 512])
                xks.append(xk)
            # Q inputs: finish before the gathers occupy the fabric
            nc.sync.dma_start(out=wq, in_=WqT[:])
            nc.sync.dma_start(out=xq, in_=xTq[:])
            nc.scalar.dma_start(out=wv, in_=WvTh[:])
            nc.scalar.dma_start(out=maskt, in_=masks[:])

            # -------- K half-projection -> two 1MB AllGathers --------
            for g in range(2):
                kgs = stg.tile([128, 4, 1024], BF, tag="kgs", name=f"kgs_{g}",
                               bufs=1)
                for scl in range(2):
                    xk = xks[2 * g + scl]
                    for ocl in range(4):
                        ps = mixp.tile([128, 512], F32, tag="mix")
                        for dc in range(DC):
                            nc.tensor.matmul(
                                ps,
                                lhsT=wk[:, dc, ocl * 128:(ocl + 1) * 128],
                                rhs=xk[:, dc, :],
                                start=(dc == 0),
                                stop=(dc == DC - 1),
                            )
                        nc.vector.tensor_copy(
                            kgs[:, ocl, scl * 512:(scl + 1) * 512], ps
                        )
                nc.scalar.dma_start(out=kg_in[g], in_=kgs)
                nc.gpsimd.collective_compute(
                    "AllGather",
                    mybir.AluOpType.bypass,
                    replica_groups=GROUPS,
                    ins=[kg_in[g]],
                    outs=[kg_out[g]],
                )

            # -------- V half-projection (resident x) -> two 1MB AllGathers --
            for g in range(2):
                vgs = stg.tile([128, 8, 512], BF, tag="vgs", name=f"vgs_{g}",
                               bufs=1)
                for scl in range(2):
                    xk = xks[2 * g + scl]
                    for sb in range(4):
                        ps = mixp.tile([128, 512], F32, tag="mix",
                                       name=f"ps2_{g}_{scl}_{sb}")
                        for dc in range(DC):
                            nc.tensor.matmul(
                                ps,
                                lhsT=xk[:, dc, sb * 128:(sb + 1) * 128],
                                rhs=wv[:, dc, :],
                                start=(dc == 0),
                                stop=(dc == DC - 1),
                            )
                        nc.vector.tensor_copy(vgs[:, scl * 4 + sb, :], ps)
                if g == 0:
                    nc.scalar.dma_start(out=vg_in[0], in_=vgs)
                    nc.gpsimd.collective_compute(
                        "AllGather",
                        mybir.AluOpType.bypass,
                        replica_groups=GROUPS,
                        ins=[vg_in[0]],
                        outs=[vg_out[0]],
                    )
                    # K groups -> SBUF on scalar; blocks only until the K
                    # gathers complete, which precede V work on the link
                    for gg in range(2):
                        for r in range(2):
                            nc.scalar.dma_start(
                                out=kts[gg][:, r * 4:(r + 1) * 4, :],
                                in_=kg_out[gg, r],
                            )
                    vgs0 = vgs
                else:
                    nc.scalar.dma_start(out=vg_in[1], in_=vgs)
                    nc.gpsimd.collective_compute(
                        "AllGather",
                        mybir.AluOpType.bypass,
                        replica_groups=GROUPS,
                        ins=[vg_in[1]],
                        outs=[vg_out[1]],
                    )
            # V groups -> SBUF on sync (its remaining work, the out stores,
            # is needed far later)
            for g in range(2):
                for r in range(2):
                    nc.sync.dma_start(
                        out=vvs[g][:, r, :, :], in_=vg_out[g, r]
                    )

            # -------- Q projection -> qt --------
            for sc in range(2):
                for oc in range(8):
                    ps = mixp.tile([128, 512], F32, tag="mix",
                                   name=f"ps0_{sc}_{oc}")
                    for dc in range(DC):
                        nc.tensor.matmul(
                            ps,
                            lhsT=wq[:, dc, oc * 128:(oc + 1) * 128],
                            rhs=xq[:, dc, sc * 512:(sc + 1) * 512],
                            start=(dc == 0),
                            stop=(dc == DC - 1),
                        )
                    nc.vector.tensor_copy(
                        qt[:, oc, sc * 512:(sc + 1) * 512], ps
                    )

            # ---------------- attention ----------------
            # pt_of[(t, kb)] -> (tile, column offset of quad t's 256 cols)
            pt_of = {}

            def kslice(kb, dc):
                return kts[kb // 8][:, dc, (kb % 8) * 128:(kb % 8 + 1) * 128]

            def emit_scores_group(tlo):
                # group = quads (tlo, tlo+1), query cols [256*tlo ...]
                qbase = tlo * 256
                Lfull = 4 * tlo + 4   # kb limit for the low quad
                Lhigh = 4 * tlo + 8   # kb limit for the high quad
                for kb in range(Lfull):
                    ps = mixp.tile([128, 512], F32, tag="mix",
                                   name=f"s5_{tlo}_{kb}")
                    for dc in range(DC):
                        nc.tensor.matmul(
                            ps,
                            lhsT=kslice(kb, dc),
                            rhs=qt[:, dc, qbase:qbase + 512],
                            start=(dc == 0),
                            stop=(dc == DC - 1),
                        )
                    pt = ptp5.tile([128, 512], BF, tag="pt5",
                                   name=f"p5_{tlo}_{kb}")
                    nc.scalar.activation(pt, ps, EXP)
                    kbr = kb - 4 * tlo
                    if kbr >= 0:
                        nc.vector.tensor_mul(
                            pt[:, 0:256], pt[:, 0:256], maskt[:, kbr, :]
                        )
                    pt_of[(tlo, kb)] = (pt, 0)
                    pt_of[(tlo + 1, kb)] = (pt, 256)
                for kb in range(Lfull, Lhigh):
                    ps = mixp.tile([128, 512], F32, tag="mix",
                                   name=f"s2_{tlo}_{kb}")
                    for dc in range(DC):
                        nc.tensor.matmul(
                            ps[:, 0:256],
                            lhsT=kslice(kb, dc),
                            rhs=qt[:, dc, qbase + 256:qbase + 512],
                            start=(dc == 0),
                            stop=(dc == DC - 1),
                        )
                    pt = ptp2.tile([128, 256], BF, tag="pt2",
                                   name=f"p2_{tlo}_{kb}")
                    nc.scalar.activation(pt, ps[:, 0:256], EXP)
                    nc.vector.tensor_mul(pt, pt, maskt[:, kb - Lfull, :])
                    pt_of[(tlo + 1, kb)] = (pt, 0)

            def emit_av(t):
                L = 4 * t + 4
                recs = []
                for j in range(2):
                    lps = psavp.tile([128, 2], F32, tag="psav", name=f"l_{t}_{j}")
                    for kb in range(L):
                        pt, off = pt_of[(t, kb)]
                        nc.tensor.matmul(
                            lps,
                            lhsT=pt[:, off + j * 128:off + (j + 1) * 128],
                            rhs=ones,
                            start=(kb == 0),
                            stop=(kb == L - 1),
                        )
                    rec = smlp.tile([128, 1], F32, tag="rec")
                    nc.vector.reciprocal(rec, lps[:, 0:1])
                    recs.append(rec)
                for j in range(2):
                    for oh in range(2):
                        avp = psavp.tile([128, 512], F32, tag="psav",
                                         name=f"av_{t}_{j}_{oh}")
                        for kb in range(L):
                            pt, off = pt_of[(t, kb)]
                            nc.tensor.matmul(
                                avp,
                                lhsT=pt[:, off + j * 128:off + (j + 1) * 128],
                                rhs=vvs[kb // 8][:, oh, kb % 8, :],
                                start=(kb == 0),
                                stop=(kb == L - 1),
                            )
                        ot = otp.tile([128, 512], F32, tag="ot",
                                      name=f"ot_{t}_{j}_{oh}")
                        nc.vector.tensor_scalar_mul(ot, avp, recs[j])
                        nc.sync.dma_start(
                            out=out[t * 256 + j * 128: t * 256 + (j + 1) * 128,
                                    oh * 512:(oh + 1) * 512],
                            in_=ot,
                        )

            # group 0 first: it only needs the first K gather group, giving
            # the second K group and the V gathers extra slack
            emit_scores_group(0)   # quads 0,1 (kb < 8)
            emit_scores_group(2)   # quads 2,3 (all of kt)
            emit_av(3)
            emit_av(2)
            emit_av(1)
            emit_av(0)
    nc.compile()
    return nc


def _query_cols(sub):
    return np.concatenate(
        [
            np.arange((4 * t + 2 * sub) * 128, (4 * t + 2 * sub + 2) * 128)
            for t in range(4)
        ]
    )


def _masks(sub):
    m = np.zeros((4, 128, 256), np.float32)
    p = np.arange(128)[:, None]
    j = np.arange(256)[None, :]
    qoff = (2 * sub + j // 128) * 128 + j % 128
    for kbr in range(4):
        m[kbr] = (kbr * 128 + p <= qoff).astype(np.float32)
    return np.ascontiguousarray(m.transpose(1, 0, 2))  # -> [128, 4, 256]


def _pmaj(a):
    """[dc*128, cols] -> partition-major [128, dc, cols]."""
    d, cols = a.shape
    return np.ascontiguousarray(a.reshape(d // 128, 128, cols).transpose(1, 0, 2))


def kernel(x, Wq, Wk, Wv, _trace=False):
    if "nc" not in _cache:
        _cache["nc"] = build_nc()
    nc = _cache["nc"]

    bf = ml_dtypes.bfloat16
    x = np.asarray(x, dtype=np.float32)
    WqT = _pmaj((np.asarray(Wq, np.float32).T * np.float32(SCALE)).astype(bf))
    WkT = np.asarray(Wk, np.float32).T.astype(bf)
    WvT = np.asarray(Wv, np.float32).T.astype(bf)

    in_maps = []
    for c in range(8):
        b, sub = c // 2, c % 2
        xT = x[b].T.astype(bf)
        in_maps.append(
            {
                "xT": _pmaj(xT),
                "xTq": _pmaj(np.ascontiguousarray(xT[:, _query_cols(sub)])),
                "WqT": WqT,
                "WkTh": _pmaj(WkT[:, sub * 512:(sub + 1) * 512]),
                "WvTh": _pmaj(WvT[:, sub * 512:(sub + 1) * 512]),
                "masks": _masks(sub).astype(bf),
            }
        )

    res = run_bass_kernel_spmd(
        nc, in_maps, core_ids=list(range(8)), trace=_trace
    )
    full = np.empty((B, S, D), np.float32)
    for c in range(8):
        b, sub = c // 2, c % 2
        full[b, _query_cols(sub)] = res.results[c]["out"]
    if _trace:
        _cache["last_result"] = res
    return full


# revision 23
# speedup vs baseline: 1.2099x; 1.0335x over previous
"""Causal single-head attention (B=4, S=2048, d=1024) on 8 TRN2 NeuronCores.

Sharding (uniform single program): core c -> batch b = c//2, subset
s = c%2. Per batch, the 16 query blocks of 128 rows are split into
quads t=0..3; core (b,s) owns blocks {4t+2s, 4t+2s+1}. Every core runs
the identical instruction stream (padded causal limit (t+1)*512 per
quad); the true causal boundary comes from per-core 0/1 mask tiles
supplied as input data.

K/V projections are tensor-parallel within each core pair: core (b,s)
computes the d_out-half s of kT and v for the whole batch; halves are
exchanged with pairwise AllGathers ([[0,1],[2,3],[4,5],[6,7]]).

v7 schedule: a tiny warmup collective absorbs the one-time collective
barrier/start delay. K projection streams over resident x chunks
(loaded once via three parallel DMA queues); each 1024-key half is
AllGathered (1 MB) as soon as it completes. V projection reuses the
resident chunks with two more 1 MB gathers. All Q-projection inputs
load before the gather window so the link sees minimal contention.
Gathered K/V land in per-group SBUF tiles (fine-grained readiness).
Attention: scores for quads 0,1 (needs only K group 0), then quads
2,3, then AV per quad in descending t so the tail is the shortest
chain. Score matmuls are 512 queries wide where the causal structure
allows.

Compute (bf16 operands, fp32 PSUM accumulate):
  scoresT[k,q] = kt.T-slices @ qT-group, p = exp(scoresT)
  (no max subtraction: |scores| <= ~2), mask, then
  out[q,:] = (pT.T @ v) / (pT.T @ ones)  -- row sums via ones-matmul.
"""
import sys

sys.path.insert(0, "/opt/trn_rl_repo")

import ml_dtypes
import numpy as np

import concourse.bass as bass  # noqa: F401
import concourse.mybir as mybir
import concourse.tile as tile
from concourse import bacc
from concourse.bass_utils import run_bass_kernel_spmd

B, S, D = 4, 2048, 1024
DC = D // 128          # 8 contraction chunks
NKB = S // 128         # 16 key blocks
SCALE = 1.0 / float(np.sqrt(D))
F32 = mybir.dt.float32
BF = mybir.dt.bfloat16
EXP = mybir.ActivationFunctionType.Exp
GROUPS = [[0, 1], [2, 3], [4, 5], [6, 7]]

_cache = {}


def build_nc():
    nc = bacc.Bacc("TRN2", target_bir_lowering=False, debug=False, num_devices=8)
    # all inputs partition-major: [128, dc, cols]
    xT = nc.dram_tensor("xT", [128, DC, S], BF, kind="ExternalInput")
    xTq = nc.dram_tensor("xTq", [128, DC, 1024], BF, kind="ExternalInput")
    WqT = nc.dram_tensor("WqT", [128, DC, D], BF, kind="ExternalInput")
    WkTh = nc.dram_tensor("WkTh", [128, DC, 512], BF, kind="ExternalInput")
    WvTh = nc.dram_tensor("WvTh", [128, DC, 512], BF, kind="ExternalInput")
    masks = nc.dram_tensor("masks", [128, 4, 256], BF, kind="ExternalInput")
    out = nc.dram_tensor("out", [1024, D], F32, kind="ExternalOutput")
    kg_in = nc.dram_tensor("kg_in", [2, 128, 4, 1024], BF)
    kg_out = nc.dram_tensor("kg_out", [2, 2, 128, 4, 1024], BF)
    vg_in = nc.dram_tensor("vg_in", [2, 128, 8, 512], BF)
    vg_out = nc.dram_tensor("vg_out", [2, 2, 128, 8, 512], BF)
    warm_in = nc.dram_tensor("warm_in", [1, 64], BF)
    warm_out = nc.dram_tensor("warm_out", [2, 64], BF)

    with tile.TileContext(nc) as tc:
        with (
            tc.tile_pool(name="w", bufs=1) as wp,
            tc.tile_pool(name="per", bufs=1) as per,
            tc.tile_pool(name="px", bufs=4) as pxp,
            tc.tile_pool(name="stg", bufs=2) as stg,
            tc.tile_pool(name="pt5", bufs=16) as ptp5,
            tc.tile_pool(name="pt2", bufs=8) as ptp2,
            tc.tile_pool(name="ot", bufs=4) as otp,
            tc.tile_pool(name="sml", bufs=4) as smlp,
            tc.tile_pool(name="mix", bufs=5, space="PSUM") as mixp,
            tc.tile_pool(name="psav", bufs=3, space="PSUM") as psavp,
        ):
            # tiny warmup AllGather (garbage data, nobody reads it): absorbs
            # the one-time collective-stream barrier + start delay
            nc.gpsimd.collective_compute(
                "AllGather",
                mybir.AluOpType.bypass,
                replica_groups=GROUPS,
                ins=[warm_in[:]],
                outs=[warm_out[:]],
            )

            # ---------------- consts + persistent ----------------
            # kt/vv split per 1024-key gather group for fine-grained readiness
            kts = [per.tile([128, DC, 1024], BF, name=f"kt{g}") for g in range(2)]
            vvs = [per.tile([128, 2, 8, 512], BF, name=f"vv{g}") for g in range(2)]
            qt = per.tile([128, DC, 1024], BF)     # qT: [d_out, 1024]
            zeros_f = per.tile([128, 2], F32)
            ones = per.tile([128, 2], BF)
            maskt = per.tile([128, 4, 256], BF)
            nc.vector.memset(zeros_f, 0.0)
            # exp(0)=1 -> also preloads the ACT exp table long before attention
            nc.scalar.activation(ones, zeros_f, EXP)

            wk = wp.tile([128, DC, 512], BF)
            wv = wp.tile([128, DC, 512], BF)
            wq = wp.tile([128, DC, D], BF)
            xq = wp.tile([128, DC, 1024], BF)
            # startup: wk on sync while xk0 arrives via the scalar queue
            # (two parallel ~125GB/s queues); gpsimd only ever triggers
            # collectives so its ring stays clear of the warmup op.
            nc.sync.dma_start(out=wk, in_=WkTh[:])
            xks = []
            for sc in range(4):
                xk = pxp.tile([128, DC, 512], BF, tag="xs", name=f"xk_{sc}")
                eng = (nc.scalar, nc.sync, nc.sync, nc.sync)[sc]
                eng.dma_start(out=xk, in_=xT[:, :, sc * 512:(sc + 1) * 512])
                xks.append(xk)
            # Q inputs: finish before the gathers occupy the fabric
            nc.sync.dma_start(out=wq, in_=WqT[:])
            nc.sync.dma_start(out=xq, in_=xTq[:])
            nc.scalar.dma_start(out=wv, in_=WvTh[:])
            nc.scalar.dma_start(out=maskt, in_=masks[:])

            # -------- K half-projection -> two 1MB AllGathers --------
            for g in range(2):
                kgs = stg.tile([128, 4, 1024], BF, tag="kgs", name=f"kgs_{g}",
                               bufs=1)
                for scl in range(2):
                    xk = xks[2 * g + scl]
                    for ocl in range(4):
                        ps = mixp.tile([128, 512], F32, tag="mix")
                        for dc in range(DC):
                            nc.tensor.matmul(
                                ps,
                                lhsT=wk[:, dc, ocl * 128:(ocl + 1) * 128],
                                rhs=xk[:, dc, :],
                                start=(dc == 0),
                                stop=(dc == DC - 1),
                            )
                        nc.vector.tensor_copy(
                            kgs[:, ocl, scl * 512:(scl + 1) * 512], ps
                        )
                nc.scalar.dma_start(out=kg_in[g], in_=kgs)
                nc.gpsimd.collective_compute(
                    "AllGather",
                    mybir.AluOpType.bypass,
                    replica_groups=GROUPS,
                    ins=[kg_in[g]],
                    outs=[kg_out[g]],
                )

            # -------- V half-projection (resident x) -> two 1MB AllGathers --
            for g in range(2):
                vgs = stg.tile([128, 8, 512], BF, tag="vgs", name=f"vgs_{g}",
                               bufs=1)
                for scl in range(2):
                    xk = xks[2 * g + scl]
                    for sb in range(4):
                        ps = mixp.tile([128, 512], F32, tag="mix",
                                       name=f"ps2_{g}_{scl}_{sb}")
                        for dc in range(DC):
                            nc.tensor.matmul(
                                ps,
                                lhsT=xk[:, dc, sb * 128:(sb + 1) * 128],
                                rhs=wv[:, dc, :],
                                start=(dc == 0),
                                stop=(dc == DC - 1),
                            )
                        nc.vector.tensor_copy(vgs[:, scl * 4 + sb, :], ps)
                if g == 0:
                    nc.scalar.dma_start(out=vg_in[0], in_=vgs)
                    nc.gpsimd.collective_compute(
                        "AllGather",
                        mybir.AluOpType.bypass,
                        replica_groups=GROUPS,
                        ins=[vg_in[0]],
                        outs=[vg_out[0]],
                    )
                    # K groups -> SBUF on scalar; blocks only until the K
                    # gathers complete, which precede V work on the link
                    for gg in range(2):
                        for r in range(2):
                            nc.scalar.dma_start(
                                out=kts[gg][:, r * 4:(r + 1) * 4, :],
                                in_=kg_out[gg, r],
                            )
                    vgs0 = vgs
                else:
                    nc.scalar.dma_start(out=vg_in[1], in_=vgs)
                    nc.gpsimd.collective_compute(
                        "AllGather",
                        mybir.AluOpType.bypass,
                        replica_groups=GROUPS,
                        ins=[vg_in[1]],
                        outs=[vg_out[1]],
                    )
            # V groups -> SBUF on sync (its remaining work, the out stores,
            # is needed far later)
            for g in range(2):
                for r in range(2):
                    nc.sync.dma_start(
                        out=vvs[g][:, r, :, :], in_=vg_out[g, r]
                    )

            # -------- Q projection -> qt --------
            for sc in range(2):
                for oc in range(8):
                    ps = mixp.tile([128, 512], F32, tag="mix",
                                   name=f"ps0_{sc}_{oc}")
                    for dc in range(DC):
                        nc.tensor.matmul(
                            ps,
                            lhsT=wq[:, dc, oc * 128:(oc + 1) * 128],
                            rhs=xq[:, dc, sc * 512:(sc + 1) * 512],
                            start=(dc == 0),
                            stop=(dc == DC - 1),
                        )
                    nc.vector.tensor_copy(
                        qt[:, oc, sc * 512:(sc + 1) * 512], ps
                    )

            # ---------------- attention ----------------
            # pt_of[(t, kb)] -> (tile, column offset of quad t's 256 cols)
            pt_of = {}

            def kslice(kb, dc):
                return kts[kb // 8][:, dc, (kb % 8) * 128:(kb % 8 + 1) * 128]

            def emit_scores_group(tlo):
                # group = quads (tlo, tlo+1), query cols [256*tlo ...]
                qbase = tlo * 256
                Lfull = 4 * tlo + 4   # kb limit for the low quad
                Lhigh = 4 * tlo + 8   # kb limit for the high quad
                for kb in range(Lfull):
                    ps = mixp.tile([128, 512], F32, tag="mix",
                                   name=f"s5_{tlo}_{kb}")
                    for dc in range(DC):
                        nc.tensor.matmul(
                            ps,
                            lhsT=kslice(kb, dc),
                            rhs=qt[:, dc, qbase:qbase + 512],
                            start=(dc == 0),
                            stop=(dc == DC - 1),
                        )
                    pt = ptp5.tile([128, 512], BF, tag="pt5",
                                   name=f"p5_{tlo}_{kb}")
                    nc.scalar.activation(pt, ps, EXP)
                    kbr = kb - 4 * tlo
                    if kbr >= 0:
                        nc.vector.tensor_mul(
                            pt[:, 0:256], pt[:, 0:256], maskt[:, kbr, :]
                        )
                    pt_of[(tlo, kb)] = (pt, 0)
                    pt_of[(tlo + 1, kb)] = (pt, 256)
                for kb in range(Lfull, Lhigh):
                    ps = mixp.tile([128, 512], F32, tag="mix",
                                   name=f"s2_{tlo}_{kb}")
                    for dc in range(DC):
                        nc.tensor.matmul(
                            ps[:, 0:256],
                            lhsT=kslice(kb, dc),
                            rhs=qt[:, dc, qbase + 256:qbase + 512],
                            start=(dc == 0),
                            stop=(dc == DC - 1),
                        )
                    pt = ptp2.tile([128, 256], BF, tag="pt2",
                                   name=f"p2_{tlo}_{kb}")
                    nc.scalar.activation(pt, ps[:, 0:256], EXP)
                    nc.vector.tensor_mul(pt, pt, maskt[:, kb - Lfull, :])
                    pt_of[(tlo + 1, kb)] = (pt, 0)

            def emit_av(t):
                L = 4 * t + 4
                recs = []
                for j in range(2):
                    lps = psavp.tile([128, 2], F32, tag="psav", name=f"l_{t}_{j}")
                    for kb in range(L):
                        pt, off = pt_of[(t, kb)]
                        nc.tensor.matmul(
                            lps,
                            lhsT=pt[:, off + j * 128:off + (j + 1) * 128],
                            rhs=ones,
                            start=(kb == 0),
                            stop=(kb == L - 1),
                        )
                    rec = smlp.tile([128, 1], F32, tag="rec")
                    nc.vector.reciprocal(rec, lps[:, 0:1])
                    recs.append(rec)
                for j in range(2):
                    for oh in range(2):
                        avp = psavp.tile([128, 512], F32, tag="psav",
                                         name=f"av_{t}_{j}_{oh}")
                        for kb in range(L):
                            pt, off = pt_of[(t, kb)]
                            nc.tensor.matmul(
                                avp,
                                lhsT=pt[:, off + j * 128:off + (j + 1) * 128],
                                rhs=vvs[kb // 8][:, oh, kb % 8, :],
                                start=(kb == 0),
                                stop=(kb == L - 1),
                            )
                        ot = otp.tile([128, 512], F32, tag="ot",
                                      name=f"ot_{t}_{j}_{oh}")
                        nc.vector.tensor_scalar_mul(ot, avp, recs[j])
                        # split output stores across two queues: the tail
                        # otherwise serializes on a single queue's drain
                        eng = nc.sync if t >= 2 else nc.scalar
                        eng.dma_start(
                            out=out[t * 256 + j * 128: t * 256 + (j + 1) * 128,
                                    oh * 512:(oh + 1) * 512],
                            in_=ot,
                        )

            # group 0 first: it only needs the first K gather group, giving
            # the second K group and the V gathers extra slack
            emit_scores_group(0)   # quads 0,1 (kb < 8)
            emit_scores_group(2)   # quads 2,3 (all of kt)
            emit_av(3)
            emit_av(2)
            emit_av(1)
            emit_av(0)
    nc.compile()
    return nc


def _query_cols(sub):
    return np.concatenate(
        [
            np.arange((4 * t + 2 * sub) * 128, (4 * t + 2 * sub + 2) * 128)
            for t in range(4)
        ]
    )


def _masks(sub):
    m = np.zeros((4, 128, 256), np.float32)
    p = np.arange(128)[:, None]
    j = np.arange(256)[None, :]
    qoff = (2 * sub + j // 128) * 128 + j % 128
    for kbr in range(4):
        m[kbr] = (kbr * 128 + p <= qoff).astype(np.float32)
    return np.ascontiguousarray(m.transpose(1, 0, 2))  # -> [128, 4, 256]


def _pmaj(a):
    """[dc*128, cols] -> partition-major [128, dc, cols]."""
    d, cols = a.shape
    return np.ascontiguousarray(a.reshape(d // 128, 128, cols).transpose(1, 0, 2))


def kernel(x, Wq, Wk, Wv, _trace=False):
    if "nc" not in _cache:
        _cache["nc"] = build_nc()
    nc = _cache["nc"]

    bf = ml_dtypes.bfloat16
    x = np.asarray(x, dtype=np.float32)
    WqT = _pmaj((np.asarray(Wq, np.float32).T * np.float32(SCALE)).astype(bf))
    WkT = np.asarray(Wk, np.float32).T.astype(bf)
    WvT = np.asarray(Wv, np.float32).T.astype(bf)

    in_maps = []
    for c in range(8):
        b, sub = c // 2, c % 2
        xT = x[b].T.astype(bf)
        in_maps.append(
            {
                "xT": _pmaj(xT),
                "xTq": _pmaj(np.ascontiguousarray(xT[:, _query_cols(sub)])),
                "WqT": WqT,
                "WkTh": _pmaj(WkT[:, sub * 512:(sub + 1) * 512]),
                "WvTh": _pmaj(WvT[:, sub * 512:(sub + 1) * 512]),
                "masks": _masks(sub).astype(bf),
            }
        )

    res = run_bass_kernel_spmd(
        nc, in_maps, core_ids=list(range(8)), trace=_trace
    )
    full = np.empty((B, S, D), np.float32)
    for c in range(8):
        b, sub = c // 2, c % 2
        full[b, _query_cols(sub)] = res.results[c]["out"]
    if _trace:
        _cache["last_result"] = res
    return full
